# revision 1
# baseline (speedup 1.0000x reference)
"""Trainium2 Bass kernel for DistanceGatedScoringFunction.

Computation (per row n of the batch):
  gl     = gate_input @ Wg + bg                       [L]
  logits = -(||gl||^2 - 2 gl @ centers.T + ||c||^2)   [E]
  logits = relu(logits @ Wgm1 + bgm1) @ Wgm2 + bgm2   [E]
  probs  = softmax(logits + gumbel)                   [E]
  eo_e   = (relu(relu(x @ We1_e + be1_e) @ We2_e + be2_e)) @ We3_e + be3_e
  out    = sigmoid(sum_e eo_e * probs_e)              [1]

Strategy: data-parallel over 8 NeuronCores (shard N), replicate params.
On-chip layout is feature-major (features on partitions, tokens on the
free dim) so every step is a plain matmul with no transposes; host-side
prep transposes the three big activations once (cheap, numpy).

Softmax is computed unnormalized: w = exp(z - mean_e(z)); the output is
sigmoid((sum eo*w) / (sum w)), which equals the softmax-weighted sum.
The mean subtraction (computed via a ones/8 matmul) keeps exp in range
(validated: max spread max-mean ~ 48 << 88 fp32 exp limit).
"""

import numpy as np

N, D, H, E, L = 100000, 256, 256, 8, 64
M_CORES = 8
NC_N = N // M_CORES     # rows per core
F = 500                 # token tile (matmul moving free dim, <=512 fp32)


def _build_nc(nc_n, f):
    """Build and compile the single-core Bass program (shared by all cores)."""
    from contextlib import ExitStack

    import concourse.bacc as bacc
    import concourse.mybir as mybir
    import concourse.tile as tile

    fp32 = mybir.dt.float32
    AF = mybir.ActivationFunctionType
    OP = mybir.AluOpType
    t_tiles = nc_n // f
    assert t_tiles * f == nc_n

    nc = bacc.Bacc("TRN2", target_bir_lowering=False, debug=False)

    # ---- DRAM I/O ----
    xs_d = nc.dram_tensor("xs", [D, nc_n], fp32, kind="ExternalInput")
    xg_d = nc.dram_tensor("xg", [D, nc_n], fp32, kind="ExternalInput")
    gm_d = nc.dram_tensor("gm", [E, nc_n], fp32, kind="ExternalInput")
    we1_d = nc.dram_tensor("we1", [128, E * 2 * H], fp32, kind="ExternalInput")
    we2_d = nc.dram_tensor("we2", [128, E * 2 * H], fp32, kind="ExternalInput")
    we3z_d = nc.dram_tensor("we3z", [128, E * 2 * E], fp32, kind="ExternalInput")
    wg_d = nc.dram_tensor("wg", [128, 2 * L], fp32, kind="ExternalInput")
    wdt_d = nc.dram_tensor("wdt", [L, E], fp32, kind="ExternalInput")
    wgm1_d = nc.dram_tensor("wgm1", [E, H], fp32, kind="ExternalInput")
    wgm2_d = nc.dram_tensor("wgm2", [128, 2 * E + 2], fp32, kind="ExternalInput")
    b128_d = nc.dram_tensor("b128", [128, 34], fp32, kind="ExternalInput")
    csm_d = nc.dram_tensor("csm", [L, 4], fp32, kind="ExternalInput")
    i8_d = nc.dram_tensor("i8", [E, E], fp32, kind="ExternalInput")
    out_d = nc.dram_tensor("out", [nc_n], fp32, kind="ExternalOutput")

    xs_r = xs_d.ap().rearrange("(c p) n -> p c n", p=128)
    xg_r = xg_d.ap().rearrange("(c p) n -> p c n", p=128)
    out_r = out_d.ap().rearrange("(t o f) -> t o f", o=1, f=f)

    with tile.TileContext(nc) as tc, ExitStack() as ctx:
        cw = ctx.enter_context(tc.tile_pool(name="cw", bufs=1))
        xin = ctx.enter_context(tc.tile_pool(name="xin", bufs=3))
        wk = ctx.enter_context(tc.tile_pool(name="wk", bufs=2))
        hp = ctx.enter_context(tc.tile_pool(name="hp", bufs=4))
        pbig = ctx.enter_context(tc.tile_pool(name="pbig", bufs=4, space="PSUM"))
        pmid = ctx.enter_context(tc.tile_pool(name="pmid", bufs=2, space="PSUM"))
        pscl = ctx.enter_context(tc.tile_pool(name="pscl", bufs=2, space="PSUM"))

        # ---- constants into SBUF (one DMA each) ----
        we1_s = cw.tile([128, E * 2 * H], fp32)
        nc.sync.dma_start(out=we1_s, in_=we1_d.ap())
        we2_s = cw.tile([128, E * 2 * H], fp32)
        nc.sync.dma_start(out=we2_s, in_=we2_d.ap())
        we3z_s = cw.tile([128, E * 2 * E], fp32)
        nc.sync.dma_start(out=we3z_s, in_=we3z_d.ap())
        wg_s = cw.tile([128, 2 * L], fp32)
        nc.sync.dma_start(out=wg_s, in_=wg_d.ap())
        wdt_s = cw.tile([L, E], fp32)
        nc.sync.dma_start(out=wdt_s, in_=wdt_d.ap())
        wgm1_s = cw.tile([E, H], fp32)
        nc.sync.dma_start(out=wgm1_s, in_=wgm1_d.ap())
        wgm2_s = cw.tile([128, 2 * E + 2], fp32)
        nc.sync.dma_start(out=wgm2_s, in_=wgm2_d.ap())
        b128_s = cw.tile([128, 34], fp32)
        nc.sync.dma_start(out=b128_s, in_=b128_d.ap())
        csm_s = cw.tile([L, 4], fp32)
        nc.sync.dma_start(out=csm_s, in_=csm_d.ap())
        i8_s = cw.tile([E, E], fp32)
        nc.sync.dma_start(out=i8_s, in_=i8_d.ap())
        wneg64 = cw.tile([L, E], fp32)
        nc.vector.memset(wneg64, -1.0)
        ones8 = cw.tile([E, 1], fp32)
        nc.vector.memset(ones8, 1.0)
        negone = cw.tile([1, E], fp32)
        nc.vector.memset(negone, -1.0)

        bg_b = csm_s[:, 0:1]            # [64,1]
        bdist_b = csm_s[0:E, 1:2]       # [8,1]
        bgm2_b = csm_s[0:E, 2:3]        # [8,1]
        be3_b = csm_s[0:E, 3:4]         # [8,1]

        def blk(e, dc, hc):
            return (e * 2 + dc) * H + hc * 128

        for t in range(t_tiles):
            n0 = t * f
            # ---- input tiles ----
            xs_t = xin.tile([128, 2, f], fp32, tag="xs")
            nc.sync.dma_start(out=xs_t, in_=xs_r[:, :, n0 : n0 + f])
            xg_t = xin.tile([128, 2, f], fp32, tag="xg")
            nc.sync.dma_start(out=xg_t, in_=xg_r[:, :, n0 : n0 + f])
            gm_t = xin.tile([E, f], fp32, tag="gm")
            nc.sync.dma_start(out=gm_t, in_=gm_d.ap()[:, n0 : n0 + f])

            # ---- gate branch: gl = xg @ Wg + bg ----
            pgl = pmid.tile([L, f], fp32, tag="pm")
            nc.tensor.matmul(pgl, wg_s[:, 0:L], xg_t[:, 0, :], start=True, stop=False)
            nc.tensor.matmul(pgl, wg_s[:, L : 2 * L], xg_t[:, 1, :], start=False, stop=True)
            gl_t = wk.tile([L, f], fp32, tag="gl")
            nc.scalar.activation(gl_t, pgl, AF.Identity, bias=bg_b)
            glsq_t = wk.tile([L, f], fp32, tag="glsq")
            nc.scalar.activation(glsq_t, pgl, AF.Square, bias=bg_b)

            # logits = 2 c.gl - ||gl||^2 - ||c||^2
            plg = pmid.tile([E, f], fp32, tag="pm")
            nc.tensor.matmul(plg, wdt_s, gl_t, start=True, stop=False)
            nc.tensor.matmul(plg, wneg64, glsq_t, start=False, stop=True)
            lg_t = wk.tile([E, f], fp32, tag="lg")
            nc.scalar.activation(lg_t, plg, AF.Identity, bias=bdist_b)

            # gating MLP layer 1 (K=8)
            hgs = []
            for hc in range(2):
                phg = pbig.tile([128, f], fp32, tag="pb")
                nc.tensor.matmul(phg, wgm1_s[:, hc * 128 : (hc + 1) * 128], lg_t,
                                 start=True, stop=True)
                hg_t = wk.tile([128, f], fp32, tag="hg", bufs=3)
                nc.scalar.activation(hg_t, phg, AF.Relu, bias=b128_s[:, 32 + hc : 33 + hc])
                hgs.append(hg_t)

            # stabilizer: mean_e(logits2_nobias) via precomputed colsum/8
            pmean = pscl.tile([1, f], fp32, tag="ps")
            nc.tensor.matmul(pmean, wgm2_s[:, 16:17], hgs[0], start=True, stop=False)
            nc.tensor.matmul(pmean, wgm2_s[:, 17:18], hgs[1], start=False, stop=True)
            mean_t = wk.tile([1, f], fp32, tag="mean")
            nc.vector.tensor_copy(mean_t, pmean)

            # layer 2 + gumbel - mean (one accumulation group)
            p1 = pmid.tile([E, f], fp32, tag="pm")
            nc.tensor.matmul(p1, wgm2_s[:, 0:E], hgs[0], start=True, stop=False)
            nc.tensor.matmul(p1, wgm2_s[:, E : 2 * E], hgs[1], start=False, stop=False)
            nc.tensor.matmul(p1, i8_s, gm_t, start=False, stop=False)
            nc.tensor.matmul(p1, negone, mean_t, start=False, stop=True)
            w_t = wk.tile([E, f], fp32, tag="w")
            nc.scalar.activation(w_t, p1, AF.Exp, bias=bgm2_b)

            # ---- expert branch ----
            peo = pmid.tile([E, f], fp32, tag="pm")
            n_eo = 0
            for e in range(E):
                h1s = []
                for hc in range(2):
                    ph = pbig.tile([128, f], fp32, tag="pb")
                    nc.tensor.matmul(ph, we1_s[:, blk(e, 0, hc) : blk(e, 0, hc) + 128],
                                     xs_t[:, 0, :], start=True, stop=False)
                    nc.tensor.matmul(ph, we1_s[:, blk(e, 1, hc) : blk(e, 1, hc) + 128],
                                     xs_t[:, 1, :], start=False, stop=True)
                    h1_t = hp.tile([128, f], fp32, tag="h1")
                    nc.scalar.activation(h1_t, ph, AF.Relu,
                                         bias=b128_s[:, e * 2 + hc : e * 2 + hc + 1])
                    h1s.append(h1_t)
                h2s = []
                for kc in range(2):
                    ph = pbig.tile([128, f], fp32, tag="pb")
                    nc.tensor.matmul(ph, we2_s[:, blk(e, 0, kc) : blk(e, 0, kc) + 128],
                                     h1s[0], start=True, stop=False)
                    nc.tensor.matmul(ph, we2_s[:, blk(e, 1, kc) : blk(e, 1, kc) + 128],
                                     h1s[1], start=False, stop=True)
                    h2_t = hp.tile([128, f], fp32, tag="h2")
                    nc.vector.tensor_scalar(
                        out=h2_t, in0=ph,
                        scalar1=b128_s[:, 16 + e * 2 + kc : 17 + e * 2 + kc],
                        scalar2=0.0, op0=OP.add, op1=OP.max)
                    h2s.append(h2_t)
                for hc in range(2):
                    nc.tensor.matmul(peo, we3z_s[:, (e * 2 + hc) * E : (e * 2 + hc + 1) * E],
                                     h2s[hc], start=(n_eo == 0), stop=(n_eo == 15))
                    n_eo += 1

            eo_t = wk.tile([E, f], fp32, tag="eo")
            nc.vector.tensor_scalar(out=eo_t, in0=peo, scalar1=be3_b, scalar2=None,
                                    op0=OP.add)

            # ---- combine: sigmoid((sum eo*w) / (sum w)) ----
            ewp_t = wk.tile([E, f], fp32, tag="ewp")
            nc.vector.tensor_mul(ewp_t, eo_t, w_t)
            pden = pscl.tile([1, f], fp32, tag="ps")
            nc.tensor.matmul(pden, ones8, w_t, start=True, stop=True)
            pnum = pscl.tile([1, f], fp32, tag="ps")
            nc.tensor.matmul(pnum, ones8, ewp_t, start=True, stop=True)
            denr_t = wk.tile([1, f], fp32, tag="denr")
            nc.vector.reciprocal(denr_t, pden)
            rat_t = wk.tile([1, f], fp32, tag="rat")
            nc.vector.tensor_mul(rat_t, pnum, denr_t)
            # sigmoid(x) = 1/(1+exp(-x)) (keeps ACT on the exp table set)
            en_t = wk.tile([1, f], fp32, tag="en")
            nc.scalar.activation(en_t, rat_t, AF.Exp, scale=-1.0)
            ep1_t = wk.tile([1, f], fp32, tag="ep1")
            nc.vector.tensor_scalar_add(ep1_t, en_t, 1.0)
            out_t = wk.tile([1, f], fp32, tag="outt")
            nc.vector.reciprocal(out_t, ep1_t)
            nc.sync.dma_start(out=out_r[t], in_=out_t)

    nc.compile()
    return nc


def _pack_weights(ins):
    """Host-side packing of parameters into SBUF-ready layouts (all fp32)."""
    f32 = np.float32
    We1, be1 = np.asarray(ins["We1"], f32), np.asarray(ins["be1"], f32)
    We2, be2 = np.asarray(ins["We2"], f32), np.asarray(ins["be2"], f32)
    We3, be3 = np.asarray(ins["We3"], f32), np.asarray(ins["be3"], f32)
    Wg, bg = np.asarray(ins["Wg"], f32), np.asarray(ins["bg"], f32)
    centers = np.asarray(ins["centers"], f32)
    Wgm1, bgm1 = np.asarray(ins["Wgm1"], f32), np.asarray(ins["bgm1"], f32)
    Wgm2, bgm2 = np.asarray(ins["Wgm2"], f32), np.asarray(ins["bgm2"], f32)

    we1_p = np.ascontiguousarray(
        We1.reshape(E, 2, 128, H).transpose(2, 0, 1, 3).reshape(128, E * 2 * H))
    we2_p = np.ascontiguousarray(
        We2.reshape(E, 2, 128, H).transpose(2, 0, 1, 3).reshape(128, E * 2 * H))
    we3z = np.zeros((128, E * 2 * E), f32)
    for e in range(E):
        for hc in range(2):
            we3z[:, (e * 2 + hc) * E + e] = We3[e, hc * 128 : (hc + 1) * 128]
    wg_p = np.ascontiguousarray(
        Wg.reshape(2, 128, L).transpose(1, 0, 2).reshape(128, 2 * L))
    wdt = np.ascontiguousarray(2.0 * centers.T)                     # [L, E]
    wgm2_p = np.zeros((128, 2 * E + 2), f32)
    wgm2_p[:, : 2 * E] = Wgm2.reshape(2, 128, E).transpose(1, 0, 2).reshape(128, 2 * E)
    wgm2_p[:, 2 * E : 2 * E + 2] = (Wgm2.sum(axis=1) / 8.0).reshape(2, 128).T
    b128 = np.zeros((128, 34), f32)
    b128[:, 0:16] = be1.reshape(E, 2, 128).transpose(2, 0, 1).reshape(128, 16)
    b128[:, 16:32] = be2.reshape(E, 2, 128).transpose(2, 0, 1).reshape(128, 16)
    b128[:, 32:34] = bgm1.reshape(2, 128).T
    csm = np.zeros((L, 4), f32)
    csm[:, 0] = bg
    csm[0:E, 1] = -(centers * centers).sum(axis=1)
    csm[0:E, 2] = bgm2
    csm[0:E, 3] = be3
    return {
        "we1": we1_p, "we2": we2_p, "we3z": we3z, "wg": wg_p, "wdt": wdt,
        "wgm1": np.ascontiguousarray(Wgm1), "wgm2": wgm2_p, "b128": b128,
        "csm": csm, "i8": np.eye(E, dtype=f32),
    }


_NC_CACHE = {}


def _get_nc(nc_n, f):
    key = (nc_n, f)
    if key not in _NC_CACHE:
        _NC_CACHE[key] = _build_nc(nc_n, f)
    return _NC_CACHE[key]


def kernel(**inputs) -> np.ndarray:
    from concourse.bass_utils import run_bass_kernel_spmd

    nc = _get_nc(NC_N, F)
    wmaps = _pack_weights(inputs)

    f32 = np.float32
    score_T = np.ascontiguousarray(np.asarray(inputs["score_input"], f32).T)
    gate_T = np.ascontiguousarray(np.asarray(inputs["gate_input"], f32).T)
    gum_T = np.ascontiguousarray(np.asarray(inputs["gumbel_noise"], f32).T)

    in_maps = []
    for c in range(M_CORES):
        s = slice(c * NC_N, (c + 1) * NC_N)
        m = dict(wmaps)
        m["xs"] = np.ascontiguousarray(score_T[:, s])
        m["xg"] = np.ascontiguousarray(gate_T[:, s])
        m["gm"] = np.ascontiguousarray(gum_T[:, s])
        in_maps.append(m)

    res = run_bass_kernel_spmd(nc, in_maps, core_ids=list(range(M_CORES)))
    out = np.concatenate([res.results[c]["out"] for c in range(M_CORES)])
    return out.reshape(N, 1).astype(np.float32)


if __name__ == "__main__":
    import jax

    with jax.default_device(jax.local_devices(backend="cpu")[0]):
        import reference

        ins = reference.setup_inputs()
        ins = {k: np.asarray(v) for k, v in ins.items()}
        expected = np.asarray(reference.reference(**ins))
    out = kernel(**ins)
    err = np.abs(out - expected).max()
    print("max abs err:", err, "rel:", err / np.abs(expected).max())


# revision 8
# speedup vs baseline: 2.6690x; 2.6690x over previous
"""Trainium2 Bass kernel for DistanceGatedScoringFunction.

Computation (per row n of the batch):
  gl     = gate_input @ Wg + bg                       [L]
  logits = -(||gl||^2 - 2 gl @ centers.T + ||c||^2)   [E]
  logits = relu(logits @ Wgm1 + bgm1) @ Wgm2 + bgm2   [E]
  probs  = softmax(logits + gumbel)                   [E]
  eo_e   = (relu(relu(x @ We1_e + be1_e) @ We2_e + be2_e)) @ We3_e + be3_e
  out    = sigmoid(sum_e eo_e * probs_e)              [1]

Strategy: data-parallel over 8 NeuronCores (shard N), replicate params.
On-chip layout is feature-major (features on partitions, tokens on the
free dim) so every step is a plain matmul with no transposes; host-side
prep transposes the three big activations once (cheap, numpy).

All matmuls use float32r (replicated fp32): 1 cycle/row at moving
dim >= 256 vs 4 cycles/row for plain fp32, at near-fp32 precision.

Softmax is computed unnormalized: w = exp(z - mean_e(z)); the output is
sigmoid((sum eo*w) / (sum w)), which equals the softmax-weighted sum.
The mean subtraction (computed via a ones/8 matmul) keeps exp in range
(validated: max spread max-mean ~ 48 << 88 fp32 exp limit).  The
divisions + final sigmoid run once per core as a [100, 125]-shaped
post-pass (a [1, 500] DVE reciprocal uses 1 of 128 lanes and costs
3.3us; batched it is ~1us total).
"""

import numpy as np

N, D, H, E, L = 100000, 256, 256, 8, 64
M_CORES = 8
NC_N = N // M_CORES     # rows per core
F = 500                 # token tile (matmul moving free dim, <=512 fp32)


def _build_nc(nc_n, f):
    """Build and compile the single-core Bass program (shared by all cores)."""
    from contextlib import ExitStack

    import concourse.bacc as bacc
    import concourse.mybir as mybir
    import concourse.tile as tile

    fp32 = mybir.dt.float32
    fr = mybir.dt.float32r
    AF = mybir.ActivationFunctionType
    OP = mybir.AluOpType
    t_tiles = nc_n // f
    assert t_tiles * f == nc_n
    # post-pass shape for the divisions/sigmoid
    PP = 100
    PJ = nc_n // PP
    assert PP * PJ == nc_n

    nc = bacc.Bacc("TRN2", target_bir_lowering=False, debug=False)

    # ---- DRAM I/O ----
    xs_d = nc.dram_tensor("xs", [D, nc_n], fr, kind="ExternalInput")
    xg_d = nc.dram_tensor("xg", [D, nc_n], fr, kind="ExternalInput")
    gm_d = nc.dram_tensor("gm", [E, nc_n], fr, kind="ExternalInput")
    we1_d = nc.dram_tensor("we1", [128, E * 2 * H], fr, kind="ExternalInput")
    we2_d = nc.dram_tensor("we2", [128, E * 2 * H], fr, kind="ExternalInput")
    we3z_d = nc.dram_tensor("we3z", [128, E * 2 * E], fr, kind="ExternalInput")
    wg_d = nc.dram_tensor("wg", [128, 2 * L], fr, kind="ExternalInput")
    wdt_d = nc.dram_tensor("wdt", [L, E], fr, kind="ExternalInput")
    wgm1_d = nc.dram_tensor("wgm1", [E, H], fr, kind="ExternalInput")
    wgm2_d = nc.dram_tensor("wgm2", [128, 2 * E + 2], fr, kind="ExternalInput")
    b128_d = nc.dram_tensor("b128", [128, 34], fp32, kind="ExternalInput")
    csm_d = nc.dram_tensor("csm", [L, 4], fp32, kind="ExternalInput")
    i8_d = nc.dram_tensor("i8", [E, E], fr, kind="ExternalInput")
    cext_d = nc.dram_tensor("cext", [L, E + 1], fr, kind="ExternalInput")
    out_d = nc.dram_tensor("out", [nc_n], fp32, kind="ExternalOutput")
    scr_d = nc.dram_tensor("scr", [2, nc_n], fp32)  # den/num bounce

    xs_r = xs_d.ap().rearrange("(c p) n -> p c n", p=128)
    xg_r = xg_d.ap().rearrange("(c p) n -> p c n", p=128)

    with tile.TileContext(nc) as tc, ExitStack() as ctx:
        cw = ctx.enter_context(tc.tile_pool(name="cw", bufs=1))
        xin = ctx.enter_context(tc.tile_pool(name="xin", bufs=3))
        wk = ctx.enter_context(tc.tile_pool(name="wk", bufs=2))
        hp = ctx.enter_context(tc.tile_pool(name="hp", bufs=4))
        pbig = ctx.enter_context(tc.tile_pool(name="pbig", bufs=4, space="PSUM"))
        pmid = ctx.enter_context(tc.tile_pool(name="pmid", bufs=4, space="PSUM"))

        # ---- constants into SBUF (one DMA each) ----
        we1_s = cw.tile([128, E * 2 * H], fr)
        nc.sync.dma_start(out=we1_s, in_=we1_d.ap())
        we2_s = cw.tile([128, E * 2 * H], fr)
        nc.sync.dma_start(out=we2_s, in_=we2_d.ap())
        we3z_s = cw.tile([128, E * 2 * E], fr)
        nc.sync.dma_start(out=we3z_s, in_=we3z_d.ap())
        wg_s = cw.tile([128, 2 * L], fr)
        nc.sync.dma_start(out=wg_s, in_=wg_d.ap())
        wdt_s = cw.tile([L, E], fr)
        nc.sync.dma_start(out=wdt_s, in_=wdt_d.ap())
        wgm1_s = cw.tile([E, H], fr)
        nc.sync.dma_start(out=wgm1_s, in_=wgm1_d.ap())
        wgm2_s = cw.tile([128, 2 * E + 2], fr)
        nc.sync.dma_start(out=wgm2_s, in_=wgm2_d.ap())
        b128_s = cw.tile([128, 34], fp32)
        nc.sync.dma_start(out=b128_s, in_=b128_d.ap())
        csm_s = cw.tile([L, 4], fp32)
        nc.sync.dma_start(out=csm_s, in_=csm_d.ap())
        i8_s = cw.tile([E, E], fr)
        nc.sync.dma_start(out=i8_s, in_=i8_d.ap())
        cext_s = cw.tile([L, E + 1], fr)
        nc.sync.dma_start(out=cext_s, in_=cext_d.ap())
        wneg64 = cext_s[:, 0:E]        # all -1
        ones8 = cext_s[0:E, E : E + 1]  # all 1
        negone = cext_s[0:1, 0:E]       # row of -1

        bg_b = csm_s[:, 0:1]            # [64,1]
        bdist_b = csm_s[0:E, 1:2]       # [8,1]
        bgm2_b = csm_s[0:E, 2:3]        # [8,1]
        be3_b = csm_s[0:E, 3:4]         # [8,1]

        def blk(e, dc, hc):
            return (e * 2 + dc) * H + hc * 128

        for t in range(t_tiles):
            n0 = t * f
            # ---- input tiles ----
            xs_t = xin.tile([128, 2, f], fr, tag="xs")
            nc.sync.dma_start(out=xs_t, in_=xs_r[:, :, n0 : n0 + f])
            xg_t = xin.tile([128, 2, f], fr, tag="xg")
            nc.sync.dma_start(out=xg_t, in_=xg_r[:, :, n0 : n0 + f])
            gm_t = xin.tile([E, f], fr, tag="gm")
            nc.sync.dma_start(out=gm_t, in_=gm_d.ap()[:, n0 : n0 + f])

            # ---- gate branch: gl = xg @ Wg + bg ----
            pgl = pmid.tile([L, f], fp32, tag="pm")
            nc.tensor.matmul(pgl, wg_s[:, 0:L], xg_t[:, 0, :], start=True, stop=False)
            nc.tensor.matmul(pgl, wg_s[:, L : 2 * L], xg_t[:, 1, :], start=False, stop=True)
            gl_t = wk.tile([L, f], fr, tag="gl")
            nc.scalar.activation(gl_t, pgl, AF.Identity, bias=bg_b)
            glsq_t = wk.tile([L, f], fr, tag="glsq")
            nc.scalar.activation(glsq_t, pgl, AF.Square, bias=bg_b)

            # logits = 2 c.gl - ||gl||^2 - ||c||^2
            plg = pmid.tile([E, f], fp32, tag="pm")
            nc.tensor.matmul(plg, wdt_s, gl_t, start=True, stop=False)
            nc.tensor.matmul(plg, wneg64, glsq_t, start=False, stop=True)
            lg_t = wk.tile([E, f], fr, tag="lg")
            nc.scalar.activation(lg_t, plg, AF.Identity, bias=bdist_b)

            # gating MLP layer 1 (K=8)
            hgs = []
            for hc in range(2):
                phg = pbig.tile([128, f], fp32, tag="pb")
                nc.tensor.matmul(phg, wgm1_s[:, hc * 128 : (hc + 1) * 128], lg_t,
                                 start=True, stop=True)
                hg_t = wk.tile([128, f], fr, tag="hg", bufs=3)
                nc.scalar.activation(hg_t, phg, AF.Relu, bias=b128_s[:, 32 + hc : 33 + hc])
                hgs.append(hg_t)

            # stabilizer: mean_e(logits2_nobias) via precomputed colsum/8
            pmean = pmid.tile([1, f], fp32, tag="pm")
            nc.tensor.matmul(pmean, wgm2_s[:, 16:17], hgs[0], start=True, stop=False)
            nc.tensor.matmul(pmean, wgm2_s[:, 17:18], hgs[1], start=False, stop=True)
            mean_t = wk.tile([1, f], fr, tag="mean")
            nc.vector.tensor_copy(mean_t, pmean)

            # layer 2 + gumbel - mean (one accumulation group)
            p1 = pmid.tile([E, f], fp32, tag="pm")
            nc.tensor.matmul(p1, wgm2_s[:, 0:E], hgs[0], start=True, stop=False)
            nc.tensor.matmul(p1, wgm2_s[:, E : 2 * E], hgs[1], start=False, stop=False)
            nc.tensor.matmul(p1, i8_s, gm_t, start=False, stop=False)
            nc.tensor.matmul(p1, negone, mean_t, start=False, stop=True)
            w_t = wk.tile([E, f], fr, tag="w")
            nc.scalar.activation(w_t, p1, AF.Exp, bias=bgm2_b)

            # ---- expert branch ----
            peo = pmid.tile([E, f], fp32, tag="pm")
            n_eo = 0
            for e in range(E):
                h1s = []
                for hc in range(2):
                    ph = pbig.tile([128, f], fp32, tag="pb")
                    nc.tensor.matmul(ph, we1_s[:, blk(e, 0, hc) : blk(e, 0, hc) + 128],
                                     xs_t[:, 0, :], start=True, stop=False)
                    nc.tensor.matmul(ph, we1_s[:, blk(e, 1, hc) : blk(e, 1, hc) + 128],
                                     xs_t[:, 1, :], start=False, stop=True)
                    h1_t = hp.tile([128, f], fr, tag="h1")
                    nc.scalar.activation(h1_t, ph, AF.Relu,
                                         bias=b128_s[:, e * 2 + hc : e * 2 + hc + 1])
                    h1s.append(h1_t)
                h2s = []
                for kc in range(2):
                    ph = pbig.tile([128, f], fp32, tag="pb")
                    nc.tensor.matmul(ph, we2_s[:, blk(e, 0, kc) : blk(e, 0, kc) + 128],
                                     h1s[0], start=True, stop=False)
                    nc.tensor.matmul(ph, we2_s[:, blk(e, 1, kc) : blk(e, 1, kc) + 128],
                                     h1s[1], start=False, stop=True)
                    h2_t = hp.tile([128, f], fr, tag="h2")
                    nc.vector.tensor_scalar(
                        out=h2_t, in0=ph,
                        scalar1=b128_s[:, 16 + e * 2 + kc : 17 + e * 2 + kc],
                        scalar2=0.0, op0=OP.add, op1=OP.max)
                    h2s.append(h2_t)
                for hc in range(2):
                    nc.tensor.matmul(peo, we3z_s[:, (e * 2 + hc) * E : (e * 2 + hc + 1) * E],
                                     h2s[hc], start=(n_eo == 0), stop=(n_eo == 15))
                    n_eo += 1

            eo_t = wk.tile([E, f], fp32, tag="eo")
            nc.vector.tensor_scalar(out=eo_t, in0=peo, scalar1=be3_b, scalar2=None,
                                    op0=OP.add)

            # ---- num/den rows for the post-pass ----
            ewp_t = wk.tile([E, f], fr, tag="ewp")
            nc.vector.tensor_mul(ewp_t, eo_t, w_t.bitcast(fp32))
            pden = pmid.tile([1, f], fp32, tag="pm")
            nc.tensor.matmul(pden, ones8, w_t, start=True, stop=True)
            pnum = pmid.tile([1, f], fp32, tag="pm")
            nc.tensor.matmul(pnum, ones8, ewp_t, start=True, stop=True)
            den_s = wk.tile([1, f], fp32, tag="dens")
            nc.scalar.activation(den_s, pden, AF.Identity)
            num_s = wk.tile([1, f], fp32, tag="nums")
            nc.vector.tensor_copy(num_s, pnum)
            nc.sync.dma_start(out=scr_d.ap()[0:1, n0 : n0 + f], in_=den_s)
            nc.sync.dma_start(out=scr_d.ap()[1:2, n0 : n0 + f], in_=num_s)

        # ---- post-pass: out = 1 / (1 + exp(-num/den)), full-width ----
        dn2 = cw.tile([PP, 2, PJ], fp32)
        nc.sync.dma_start(out=dn2, in_=scr_d.ap().rearrange("c (p j) -> p c j", p=PP))
        denr2 = cw.tile([PP, PJ], fp32)
        nc.vector.reciprocal(denr2, dn2[:, 0, :])
        rat2 = cw.tile([PP, PJ], fp32)
        nc.vector.tensor_mul(rat2, dn2[:, 1, :], denr2)
        en2 = cw.tile([PP, PJ], fp32)
        nc.scalar.activation(en2, rat2, AF.Exp, scale=-1.0)
        ep2 = cw.tile([PP, PJ], fp32)
        nc.vector.tensor_scalar_add(ep2, en2, 1.0)
        outp = cw.tile([PP, PJ], fp32)
        nc.vector.reciprocal(outp, ep2)
        nc.sync.dma_start(out=out_d.ap().rearrange("(p j) -> p j", p=PP), in_=outp)

    nc.compile()
    return nc


def _pack_weights(ins):
    """Host-side packing of parameters into SBUF-ready layouts (all fp32)."""
    f32 = np.float32
    We1, be1 = np.asarray(ins["We1"], f32), np.asarray(ins["be1"], f32)
    We2, be2 = np.asarray(ins["We2"], f32), np.asarray(ins["be2"], f32)
    We3, be3 = np.asarray(ins["We3"], f32), np.asarray(ins["be3"], f32)
    Wg, bg = np.asarray(ins["Wg"], f32), np.asarray(ins["bg"], f32)
    centers = np.asarray(ins["centers"], f32)
    Wgm1, bgm1 = np.asarray(ins["Wgm1"], f32), np.asarray(ins["bgm1"], f32)
    Wgm2, bgm2 = np.asarray(ins["Wgm2"], f32), np.asarray(ins["bgm2"], f32)

    we1_p = np.ascontiguousarray(
        We1.reshape(E, 2, 128, H).transpose(2, 0, 1, 3).reshape(128, E * 2 * H))
    we2_p = np.ascontiguousarray(
        We2.reshape(E, 2, 128, H).transpose(2, 0, 1, 3).reshape(128, E * 2 * H))
    we3z = np.zeros((128, E * 2 * E), f32)
    for e in range(E):
        for hc in range(2):
            we3z[:, (e * 2 + hc) * E + e] = We3[e, hc * 128 : (hc + 1) * 128]
    wg_p = np.ascontiguousarray(
        Wg.reshape(2, 128, L).transpose(1, 0, 2).reshape(128, 2 * L))
    wdt = np.ascontiguousarray(2.0 * centers.T)                     # [L, E]
    wgm2_p = np.zeros((128, 2 * E + 2), f32)
    wgm2_p[:, : 2 * E] = Wgm2.reshape(2, 128, E).transpose(1, 0, 2).reshape(128, 2 * E)
    wgm2_p[:, 2 * E : 2 * E + 2] = (Wgm2.sum(axis=1) / 8.0).reshape(2, 128).T
    b128 = np.zeros((128, 34), f32)
    b128[:, 0:16] = be1.reshape(E, 2, 128).transpose(2, 0, 1).reshape(128, 16)
    b128[:, 16:32] = be2.reshape(E, 2, 128).transpose(2, 0, 1).reshape(128, 16)
    b128[:, 32:34] = bgm1.reshape(2, 128).T
    csm = np.zeros((L, 4), f32)
    csm[:, 0] = bg
    csm[0:E, 1] = -(centers * centers).sum(axis=1)
    csm[0:E, 2] = bgm2
    csm[0:E, 3] = be3
    cext = np.full((L, E + 1), -1.0, f32)
    cext[:, E] = 1.0
    return {
        "we1": we1_p, "we2": we2_p, "we3z": we3z, "wg": wg_p, "wdt": wdt,
        "wgm1": np.ascontiguousarray(Wgm1), "wgm2": wgm2_p, "b128": b128,
        "csm": csm, "i8": np.eye(E, dtype=f32), "cext": cext,
    }


_NC_CACHE = {}


def _get_nc(nc_n, f):
    key = (nc_n, f)
    if key not in _NC_CACHE:
        _NC_CACHE[key] = _build_nc(nc_n, f)
    return _NC_CACHE[key]


def kernel(**inputs) -> np.ndarray:
    from concourse.bass_utils import run_bass_kernel_spmd

    nc = _get_nc(NC_N, F)
    wmaps = _pack_weights(inputs)

    f32 = np.float32
    score_T = np.ascontiguousarray(np.asarray(inputs["score_input"], f32).T)
    gate_T = np.ascontiguousarray(np.asarray(inputs["gate_input"], f32).T)
    gum_T = np.ascontiguousarray(np.asarray(inputs["gumbel_noise"], f32).T)

    in_maps = []
    for c in range(M_CORES):
        s = slice(c * NC_N, (c + 1) * NC_N)
        m = dict(wmaps)
        m["xs"] = np.ascontiguousarray(score_T[:, s])
        m["xg"] = np.ascontiguousarray(gate_T[:, s])
        m["gm"] = np.ascontiguousarray(gum_T[:, s])
        in_maps.append(m)

    res = run_bass_kernel_spmd(nc, in_maps, core_ids=list(range(M_CORES)))
    out = np.concatenate([res.results[c]["out"] for c in range(M_CORES)])
    return out.reshape(N, 1).astype(np.float32)


if __name__ == "__main__":
    import jax

    with jax.default_device(jax.local_devices(backend="cpu")[0]):
        import reference

        ins = reference.setup_inputs()
        ins = {k: np.asarray(v) for k, v in ins.items()}
        expected = np.asarray(reference.reference(**ins))
    out = kernel(**ins)
    err = np.abs(out - expected).max()
    print("max abs err:", err, "rel:", err / np.abs(expected).max())


# revision 11
# speedup vs baseline: 3.4892x; 1.3073x over previous
"""Trainium2 Bass kernel for DistanceGatedScoringFunction.

Computation (per row n of the batch):
  gl     = gate_input @ Wg + bg                       [L]
  logits = -(||gl||^2 - 2 gl @ centers.T + ||c||^2)   [E]
  logits = relu(logits @ Wgm1 + bgm1) @ Wgm2 + bgm2   [E]
  probs  = softmax(logits + gumbel)                   [E]
  eo_e   = (relu(relu(x @ We1_e + be1_e) @ We2_e + be2_e)) @ We3_e + be3_e
  out    = sigmoid(sum_e eo_e * probs_e)              [1]

Strategy: data-parallel over 8 NeuronCores (shard N), replicate params.
On-chip layout is feature-major (features on partitions, tokens on the
free dim) so every step is a plain matmul with no transposes; host-side
prep transposes the three big activations once (cheap, numpy).

All matmuls use float32r (replicated fp32): 1 cycle/row at moving
dim >= 256 vs 4 cycles/row for plain fp32, at near-fp32 precision.

Softmax is computed unnormalized: w = exp(z - mean_e(z)); the output is
sigmoid((sum eo*w) / (sum w)), which equals the softmax-weighted sum.
The mean subtraction (computed via a ones/8 matmul) keeps exp in range
(validated: max spread max-mean ~ 48 << 88 fp32 exp limit).  The
divisions + final sigmoid run once per core as a [100, 125]-shaped
post-pass (a [1, 500] DVE reciprocal uses 1 of 128 lanes and costs
3.3us; batched it is ~1us total).
"""

import numpy as np

N, D, H, E, L = 100000, 256, 256, 8, 64
M_CORES = 8
NC_N = N // M_CORES     # rows per core
F = 500                 # token tile (matmul moving free dim, <=512 fp32)


def _build_nc(nc_n, f):
    """Build and compile the single-core Bass program (shared by all cores)."""
    from contextlib import ExitStack

    import concourse.bacc as bacc
    import concourse.mybir as mybir
    import concourse.tile as tile

    fp32 = mybir.dt.float32
    fr = mybir.dt.float32r
    f16 = mybir.dt.float16
    AF = mybir.ActivationFunctionType
    OP = mybir.AluOpType
    t_tiles = nc_n // f
    assert t_tiles * f == nc_n
    # post-pass shape for the divisions/sigmoid
    PP = 100
    PJ = nc_n // PP
    assert PP * PJ == nc_n

    nc = bacc.Bacc("TRN2", target_bir_lowering=False, debug=False)

    # ---- DRAM I/O ----
    xs_d = nc.dram_tensor("xs", [D, nc_n], f16, kind="ExternalInput")
    xg_d = nc.dram_tensor("xg", [D, nc_n], fr, kind="ExternalInput")
    gm_d = nc.dram_tensor("gm", [E, nc_n], fr, kind="ExternalInput")
    we1_d = nc.dram_tensor("we1", [128, E * 2 * H], f16, kind="ExternalInput")
    we2_d = nc.dram_tensor("we2", [128, E * 2 * H], f16, kind="ExternalInput")
    we3z_d = nc.dram_tensor("we3z", [128, E * 2 * E], f16, kind="ExternalInput")
    wg_d = nc.dram_tensor("wg", [128, 2 * L], fr, kind="ExternalInput")
    wdt_d = nc.dram_tensor("wdt", [L, E], fr, kind="ExternalInput")
    wgm1_d = nc.dram_tensor("wgm1", [E, H], fr, kind="ExternalInput")
    wgm2_d = nc.dram_tensor("wgm2", [128, 2 * E + 2], fr, kind="ExternalInput")
    b128_d = nc.dram_tensor("b128", [128, 34], fp32, kind="ExternalInput")
    csm_d = nc.dram_tensor("csm", [L, 4], fp32, kind="ExternalInput")
    i8_d = nc.dram_tensor("i8", [E, E], fr, kind="ExternalInput")
    cext_d = nc.dram_tensor("cext", [L, E + 1], fr, kind="ExternalInput")
    out_d = nc.dram_tensor("out", [nc_n], fp32, kind="ExternalOutput")
    scr_d = nc.dram_tensor("scr", [2, nc_n], fp32)  # den/num bounce

    xs_r = xs_d.ap().rearrange("(c p) n -> p c n", p=128)
    xg_r = xg_d.ap().rearrange("(c p) n -> p c n", p=128)

    with tile.TileContext(nc) as tc, ExitStack() as ctx:
        cw = ctx.enter_context(tc.tile_pool(name="cw", bufs=1))
        xin = ctx.enter_context(tc.tile_pool(name="xin", bufs=3))
        wk = ctx.enter_context(tc.tile_pool(name="wk", bufs=2))
        hp = ctx.enter_context(tc.tile_pool(name="hp", bufs=4))
        pbig = ctx.enter_context(tc.tile_pool(name="pbig", bufs=4, space="PSUM"))
        pmid = ctx.enter_context(tc.tile_pool(name="pmid", bufs=4, space="PSUM"))

        # ---- constants into SBUF (one DMA each) ----
        we1_s = cw.tile([128, E * 2 * H], f16)
        nc.sync.dma_start(out=we1_s, in_=we1_d.ap())
        we2_s = cw.tile([128, E * 2 * H], f16)
        nc.sync.dma_start(out=we2_s, in_=we2_d.ap())
        we3z_s = cw.tile([128, E * 2 * E], f16)
        nc.sync.dma_start(out=we3z_s, in_=we3z_d.ap())
        wg_s = cw.tile([128, 2 * L], fr)
        nc.sync.dma_start(out=wg_s, in_=wg_d.ap())
        wdt_s = cw.tile([L, E], fr)
        nc.sync.dma_start(out=wdt_s, in_=wdt_d.ap())
        wgm1_s = cw.tile([E, H], fr)
        nc.sync.dma_start(out=wgm1_s, in_=wgm1_d.ap())
        wgm2_s = cw.tile([128, 2 * E + 2], fr)
        nc.sync.dma_start(out=wgm2_s, in_=wgm2_d.ap())
        b128_s = cw.tile([128, 34], fp32)
        nc.sync.dma_start(out=b128_s, in_=b128_d.ap())
        csm_s = cw.tile([L, 4], fp32)
        nc.sync.dma_start(out=csm_s, in_=csm_d.ap())
        i8_s = cw.tile([E, E], fr)
        nc.sync.dma_start(out=i8_s, in_=i8_d.ap())
        cext_s = cw.tile([L, E + 1], fr)
        nc.sync.dma_start(out=cext_s, in_=cext_d.ap())
        wneg64 = cext_s[:, 0:E]        # all -1
        ones8 = cext_s[0:E, E : E + 1]  # all 1
        negone = cext_s[0:1, 0:E]       # row of -1

        bg_b = csm_s[:, 0:1]            # [64,1]
        bdist_b = csm_s[0:E, 1:2]       # [8,1]
        bgm2_b = csm_s[0:E, 2:3]        # [8,1]
        be3_b = csm_s[0:E, 3:4]         # [8,1]

        def blk(e, dc, hc):
            return (e * 2 + dc) * H + hc * 128

        for t in range(t_tiles):
            n0 = t * f
            # ---- input tiles ----
            xs_t = xin.tile([128, 2, f], f16, tag="xs", name="xs_t")
            nc.sync.dma_start(out=xs_t, in_=xs_r[:, :, n0 : n0 + f])
            xg_t = xin.tile([128, 2, f], fr, tag="xg", name="xg_t")
            nc.sync.dma_start(out=xg_t, in_=xg_r[:, :, n0 : n0 + f])
            gm_t = xin.tile([E, f], fr, tag="gm", name="gm_t")
            nc.sync.dma_start(out=gm_t, in_=gm_d.ap()[:, n0 : n0 + f])

            # Gating chain, split into stages emitted between expert blocks so
            # the PE always has expert matmuls to run while the chain's
            # ACT/DVE steps drain (keeps HAM warm).
            gs = {}

            def g_gate():
                pgl = pmid.tile([L, f], fp32, tag="pm", name="pgl")
                nc.tensor.matmul(pgl, wg_s[:, 0:L], xg_t[:, 0, :], start=True, stop=False)
                nc.tensor.matmul(pgl, wg_s[:, L : 2 * L], xg_t[:, 1, :], start=False, stop=True)
                gl_t = wk.tile([L, f], fr, tag="gl", name="gl_t")
                nc.scalar.activation(gl_t, pgl, AF.Identity, bias=bg_b)
                glsq_t = wk.tile([L, f], fr, tag="glsq", name="glsq_t")
                nc.scalar.activation(glsq_t, pgl, AF.Square, bias=bg_b)
                gs["gl"], gs["glsq"] = gl_t, glsq_t

            def g_dist():
                # logits(+LG_SHIFT) = 2 c.gl - ||gl||^2 - ||c||^2 + LG_SHIFT
                plg = pmid.tile([E, f], fp32, tag="pm", name="plg")
                nc.tensor.matmul(plg, wdt_s, gs["gl"], start=True, stop=False)
                nc.tensor.matmul(plg, wneg64, gs["glsq"], start=False, stop=True)
                lg_t = wk.tile([E, f], fr, tag="lg", name="lg_t")
                nc.scalar.activation(lg_t, plg, AF.Identity, bias=bdist_b)
                gs["lg"] = lg_t

            def g_mlp1():
                hgs = []
                for hc in range(2):
                    phg = pbig.tile([128, f], fp32, tag="pb", name="phg")
                    nc.tensor.matmul(phg, wgm1_s[:, hc * 128 : (hc + 1) * 128], gs["lg"],
                                     start=True, stop=True)
                    hg_t = wk.tile([128, f], fr, tag="hg", bufs=3, name="hg_t")
                    nc.scalar.activation(hg_t, phg, AF.Relu,
                                         bias=b128_s[:, 32 + hc : 33 + hc])
                    hgs.append(hg_t)
                gs["hg"] = hgs

            def g_mean():
                pmean = pmid.tile([1, f], fp32, tag="pm", name="pmean")
                nc.tensor.matmul(pmean, wgm2_s[:, 16:17], gs["hg"][0], start=True, stop=False)
                nc.tensor.matmul(pmean, wgm2_s[:, 17:18], gs["hg"][1], start=False, stop=True)
                mean_t = wk.tile([1, f], fr, tag="mean", name="mean_t")
                nc.vector.tensor_copy(mean_t, pmean)
                gs["mean"] = mean_t

            def g_mlp2():
                p1 = pmid.tile([E, f], fp32, tag="pm", name="p1")
                nc.tensor.matmul(p1, wgm2_s[:, 0:E], gs["hg"][0], start=True, stop=False)
                nc.tensor.matmul(p1, wgm2_s[:, E : 2 * E], gs["hg"][1], start=False, stop=False)
                nc.tensor.matmul(p1, i8_s, gm_t, start=False, stop=False)
                nc.tensor.matmul(p1, negone, gs["mean"], start=False, stop=True)
                w_t = wk.tile([E, f], fr, tag="w", name="w_t")
                nc.scalar.activation(w_t, p1, AF.Exp, bias=bgm2_b)
                gs["w"] = w_t

            stages = [g_gate, g_dist, g_mlp1, g_mean, g_mlp2]

            # ---- expert branch (fp16), gating stages interleaved ----
            peo = pmid.tile([E, f], fp32, tag="pm", name="peo")
            n_eo = 0
            for e in range(E):
                h1s = []
                for hc in range(2):
                    ph = pbig.tile([128, f], fp32, tag="pb", name="ph1")
                    nc.tensor.matmul(ph, we1_s[:, blk(e, 0, hc) : blk(e, 0, hc) + 128],
                                     xs_t[:, 0, :], start=True, stop=False)
                    nc.tensor.matmul(ph, we1_s[:, blk(e, 1, hc) : blk(e, 1, hc) + 128],
                                     xs_t[:, 1, :], start=False, stop=True)
                    h1_t = hp.tile([128, f], f16, tag="h1", name="h1_t")
                    nc.scalar.activation(h1_t, ph, AF.Relu,
                                         bias=b128_s[:, e * 2 + hc : e * 2 + hc + 1])
                    h1s.append(h1_t)
                h2s = []
                for kc in range(2):
                    ph = pbig.tile([128, f], fp32, tag="pb", name="ph2")
                    nc.tensor.matmul(ph, we2_s[:, blk(e, 0, kc) : blk(e, 0, kc) + 128],
                                     h1s[0], start=True, stop=False)
                    nc.tensor.matmul(ph, we2_s[:, blk(e, 1, kc) : blk(e, 1, kc) + 128],
                                     h1s[1], start=False, stop=True)
                    h2_t = hp.tile([128, f], f16, tag="h2", name="h2_t")
                    nc.vector.tensor_scalar(
                        out=h2_t, in0=ph,
                        scalar1=b128_s[:, 16 + e * 2 + kc : 17 + e * 2 + kc],
                        scalar2=0.0, op0=OP.add, op1=OP.max)
                    h2s.append(h2_t)
                for hc in range(2):
                    nc.tensor.matmul(peo, we3z_s[:, (e * 2 + hc) * E : (e * 2 + hc + 1) * E],
                                     h2s[hc], start=(n_eo == 0), stop=(n_eo == 15))
                    n_eo += 1
                if e < len(stages):
                    stages[e]()

            eo_t = wk.tile([E, f], fp32, tag="eo", name="eo_t")
            nc.vector.tensor_scalar(out=eo_t, in0=peo, scalar1=be3_b, scalar2=None,
                                    op0=OP.add)

            # ---- num/den rows for the post-pass ----
            ewp_t = wk.tile([E, f], fr, tag="ewp", name="ewp_t")
            nc.vector.tensor_mul(ewp_t, eo_t, gs["w"].bitcast(fp32))
            pden = pmid.tile([1, f], fp32, tag="pm", name="pden")
            nc.tensor.matmul(pden, ones8, gs["w"], start=True, stop=True)
            pnum = pmid.tile([1, f], fp32, tag="pm", name="pnum")
            nc.tensor.matmul(pnum, ones8, ewp_t, start=True, stop=True)
            den_s = wk.tile([1, f], fp32, tag="dens", name="den_s")
            nc.scalar.activation(den_s, pden, AF.Identity)
            num_s = wk.tile([1, f], fp32, tag="nums", name="num_s")
            nc.vector.tensor_copy(num_s, pnum)
            nc.sync.dma_start(out=scr_d.ap()[0:1, n0 : n0 + f], in_=den_s)
            nc.sync.dma_start(out=scr_d.ap()[1:2, n0 : n0 + f], in_=num_s)

        # ---- post-pass: out = 1 / (1 + exp(-num/den)), full-width ----
        dn2 = cw.tile([PP, 2, PJ], fp32)
        nc.sync.dma_start(out=dn2, in_=scr_d.ap().rearrange("c (p j) -> p c j", p=PP))
        denr2 = cw.tile([PP, PJ], fp32)
        nc.vector.reciprocal(denr2, dn2[:, 0, :])
        rat2 = cw.tile([PP, PJ], fp32)
        nc.vector.tensor_mul(rat2, dn2[:, 1, :], denr2)
        en2 = cw.tile([PP, PJ], fp32)
        nc.scalar.activation(en2, rat2, AF.Exp, scale=-1.0)
        ep2 = cw.tile([PP, PJ], fp32)
        nc.vector.tensor_scalar_add(ep2, en2, 1.0)
        outp = cw.tile([PP, PJ], fp32)
        nc.vector.reciprocal(outp, ep2)
        nc.sync.dma_start(out=out_d.ap().rearrange("(p j) -> p j", p=PP), in_=outp)

    nc.compile()
    return nc


def _pack_weights(ins):
    """Host-side packing of parameters into SBUF-ready layouts (all fp32)."""
    f32 = np.float32
    We1, be1 = np.asarray(ins["We1"], f32), np.asarray(ins["be1"], f32)
    We2, be2 = np.asarray(ins["We2"], f32), np.asarray(ins["be2"], f32)
    We3, be3 = np.asarray(ins["We3"], f32), np.asarray(ins["be3"], f32)
    Wg, bg = np.asarray(ins["Wg"], f32), np.asarray(ins["bg"], f32)
    centers = np.asarray(ins["centers"], f32)
    Wgm1, bgm1 = np.asarray(ins["Wgm1"], f32), np.asarray(ins["bgm1"], f32)
    Wgm2, bgm2 = np.asarray(ins["Wgm2"], f32), np.asarray(ins["bgm2"], f32)

    LG_SHIFT = np.float32(90.0)  # recenters the large dist-logits near 0
    we1_p = np.ascontiguousarray(
        We1.reshape(E, 2, 128, H).transpose(2, 0, 1, 3).reshape(128, E * 2 * H)
    ).astype(np.float16)
    we2_p = np.ascontiguousarray(
        We2.reshape(E, 2, 128, H).transpose(2, 0, 1, 3).reshape(128, E * 2 * H)
    ).astype(np.float16)
    we3z = np.zeros((128, E * 2 * E), np.float16)
    for e in range(E):
        for hc in range(2):
            we3z[:, (e * 2 + hc) * E + e] = We3[e, hc * 128 : (hc + 1) * 128]
    wg_p = np.ascontiguousarray(
        Wg.reshape(2, 128, L).transpose(1, 0, 2).reshape(128, 2 * L))
    wdt = np.ascontiguousarray(2.0 * centers.T)                     # [L, E]
    wgm2_p = np.zeros((128, 2 * E + 2), f32)
    wgm2_p[:, : 2 * E] = Wgm2.reshape(2, 128, E).transpose(1, 0, 2).reshape(128, 2 * E)
    wgm2_p[:, 2 * E : 2 * E + 2] = (Wgm2.sum(axis=1) / 8.0).reshape(2, 128).T
    b128 = np.zeros((128, 34), f32)
    b128[:, 0:16] = be1.reshape(E, 2, 128).transpose(2, 0, 1).reshape(128, 16)
    b128[:, 16:32] = be2.reshape(E, 2, 128).transpose(2, 0, 1).reshape(128, 16)
    b128[:, 32:34] = (bgm1 - LG_SHIFT * Wgm1.sum(axis=0)).reshape(2, 128).T
    csm = np.zeros((L, 4), f32)
    csm[:, 0] = bg
    csm[0:E, 1] = -(centers * centers).sum(axis=1) + LG_SHIFT
    csm[0:E, 2] = bgm2
    csm[0:E, 3] = be3
    cext = np.full((L, E + 1), -1.0, f32)
    cext[:, E] = 1.0
    return {
        "we1": we1_p, "we2": we2_p, "we3z": we3z, "wg": wg_p, "wdt": wdt,
        "wgm1": np.ascontiguousarray(Wgm1), "wgm2": wgm2_p, "b128": b128,
        "csm": csm, "i8": np.eye(E, dtype=f32), "cext": cext,
    }


_NC_CACHE = {}


def _get_nc(nc_n, f):
    key = (nc_n, f)
    if key not in _NC_CACHE:
        _NC_CACHE[key] = _build_nc(nc_n, f)
    return _NC_CACHE[key]


def kernel(**inputs) -> np.ndarray:
    from concourse.bass_utils import run_bass_kernel_spmd

    nc = _get_nc(NC_N, F)
    wmaps = _pack_weights(inputs)

    f32 = np.float32
    score_T = np.ascontiguousarray(np.asarray(inputs["score_input"], f32).T.astype(np.float16))
    gate_T = np.ascontiguousarray(np.asarray(inputs["gate_input"], f32).T)
    gum_T = np.ascontiguousarray(np.asarray(inputs["gumbel_noise"], f32).T)

    in_maps = []
    for c in range(M_CORES):
        s = slice(c * NC_N, (c + 1) * NC_N)
        m = dict(wmaps)
        m["xs"] = np.ascontiguousarray(score_T[:, s])
        m["xg"] = np.ascontiguousarray(gate_T[:, s])
        m["gm"] = np.ascontiguousarray(gum_T[:, s])
        in_maps.append(m)

    res = run_bass_kernel_spmd(nc, in_maps, core_ids=list(range(M_CORES)))
    out = np.concatenate([res.results[c]["out"] for c in range(M_CORES)])
    return out.reshape(N, 1).astype(np.float32)


if __name__ == "__main__":
    import jax

    with jax.default_device(jax.local_devices(backend="cpu")[0]):
        import reference

        ins = reference.setup_inputs()
        ins = {k: np.asarray(v) for k, v in ins.items()}
        expected = np.asarray(reference.reference(**ins))
    out = kernel(**ins)
    err = np.abs(out - expected).max()
    print("max abs err:", err, "rel:", err / np.abs(expected).max())
